# revision 11
# baseline (speedup 1.0000x reference)
"""Trainium2 Bass kernel for nn_D_GA_1812476199112 (maxpool -> 16-head
attention over 1024 tokens -> proj -> batchnorm -> maxunpool).

Sharding: data-parallel over batch B=8, one batch element per NeuronCore.
Everything is local per core; no collectives.

Per-core pipeline, "folded" layout [128, 2048]: image rows h<32 on
partitions 0-63 (channels), rows h>=32 on partitions 64-127. This halves
the free-size (and thus engine time) of every elementwise op, and makes
the two query halves land on the two partition halves for the QKV prep
matmuls (weights are host-duplicated across both partition halves).

  1. MaxPool2d(2,2) via strided DVE max ops; argmax becomes first-match
     masks (is_equal + not-found chain) interleaved into attention slack.
  2. Q^T/K^T strip-packed (head h=4sg+c at partitions 32c..32c+4) via
     host-permuted zero-padded weights; moving data is f32r so prep
     matmuls run at 1 cycle/row. Softmax scale 0.5 folded into wq.
  3. Scores S^T [keys, queries] per (head, kt, qh): 512-col fp32r
     matmuls into 1-bank PSUM slots (6 rotating).
  4. Softmax exp split across THREE engines by a greedy planner:
     ACT native Exp, DVE and GpSimd use the Schraudolph bit-trick
     (i32 = round(A*s + B) bitcast to float ~= exp(s), max ripple 3%;
     the softmax ratio + value-averaging cancels it to ~1e-3 end-to-end).
  5. AV flipped: stationary = exp chunk [128 keys, 128 queries] (f32r
     bitcast), moving = V~ [128, 5] bf16 (ones col accumulates softmax
     denominators) -> output free size 5, so all 2048 AV matmuls cost
     ~5 cycles each instead of 512. Accumulators [128, 4qc, 80] per query
     half, one PSUM bank each, start/stop per (qchunk, head) group.
  6. Tail per qchunk: DVE reciprocal (strided denom read) -> normalize
     with broadcast multiply -> DVE 32x32 block transpose -> block-wise
     proj matmuls (stationary = host-replicated wprojP, tile_position
     routes output to the folded partition half) -> BN as one DVE
     tensor_scalar with per-partition scale/bias -> unpool via 4 masked
     multiplies [128, 512] (DVE/GpSimd) -> 4 output DMA quarters.
"""
import numpy as np

DIM = 64
HEAD_DIM = 4
NUM_HEADS = 16
B = 8
H = W = 64
HP = WP = 32
N = HP * WP          # 1024 tokens
NKT = 8              # key tiles of 128
BN_EPS = 1e-5

# Schraudolph exp constants (bf16 flavor): i16 = round(A*x + B);
# bitcast the int16 bit pattern as bfloat16 -> ~exp(x), max ripple ~3%.
SCH_A = float(128.0 / np.log(2.0))
SCH_B = float(127 * 128 - 366393.0 / 65536.0)

_CACHE = {}


def _plan_engines():
    """Greedy engine assignment for the 128 exp chunks (one per
    (sg, kt, c) pair of 512-column score units). Only ACT and DVE can
    read PSUM; GpSimd gets the masks/unpool instead."""
    cost = {"A": 1070.0, "D": 1237.0}
    # head starts: ACT pays the Exp table load + finishes its prep
    # copies late; DVE finishes its copies + vt work late.
    ready = {"A": 5000.0, "D": 5500.0}
    assign = []
    mask_jobs = {12 * i + 6: 640.0 for i in range(10)}  # 7 of 10 land on DVE
    for j in range(128):
        if j in mask_jobs and (j // 12) not in (5, 7, 9):
            ready["D"] += 640.0
        best = min(("A", "D"), key=lambda e: ready[e] + cost[e])
        ready[best] += cost[best]
        assign.append(best)
    return assign


def _build_program():
    import concourse.bass as bass
    import concourse.mybir as mybir
    import concourse.tile as tile
    from concourse import bacc

    f32 = mybir.dt.float32
    f32r = mybir.dt.float32r
    i16 = mybir.dt.int16
    bf16 = mybir.dt.bfloat16
    AF = mybir.ActivationFunctionType
    OP = mybir.AluOpType

    nc = bacc.Bacc("TRN2", debug=False)

    x_d = nc.dram_tensor("x", [128, 2048], f32, kind="ExternalInput").ap()
    wb_d = nc.dram_tensor("wb", [128, 1218], f32r, kind="ExternalInput").ap()
    out_d = nc.dram_tensor("out", [128, 2048], f32, kind="ExternalOutput").ap()

    assign = _plan_engines()

    with tile.TileContext(nc) as tc:
        with tc.tile_pool(name="sg1", bufs=1) as sg1:
            # ---------------- loads ----------------
            x_sb = sg1.tile([128, 2048], f32)
            nc.sync.dma_start(out=x_sb[0:64, 0:1024], in_=x_d[0:64, 0:1024])
            nc.sync.dma_start(out=x_sb[64:128, 0:1024], in_=x_d[64:128, 0:1024])
            nc.sync.dma_start(out=x_sb[0:64, 1024:2048], in_=x_d[0:64, 1024:2048])
            nc.sync.dma_start(out=x_sb[64:128, 1024:2048], in_=x_d[64:128, 1024:2048])
            wb_sb = sg1.tile([128, 1218], f32r)
            nc.sync.dma_start(out=wb_sb, in_=wb_d)
            wbr = wb_sb
            wqp = [wbr[:, 128 * sg:128 * sg + 128] for sg in range(4)]
            wkp = [wbr[:, 512 + 128 * sg:512 + 128 * sg + 128] for sg in range(4)]
            wv = wbr[:, 1024:1088]
            wprojP = wbr[:, 1088:1216]
            bns = wb_sb.bitcast(f32)[:, 1216:1217]
            bnb = wb_sb.bitcast(f32)[:, 1217:1218]

            # warm tiles + vt memsets during the x DMA window
            dummy_bf = sg1.tile([64, 512], bf16)
            nc.vector.memset(dummy_bf, 1.0)
            vt = [sg1.tile([128, 16, 5], bf16, tag=f"vt{kt}", name=f"vt{kt}")
                  for kt in range(NKT)]
            for kt in range(NKT):
                nc.vector.memset(vt[kt], 1.0)

            # ---------------- maxpool (2 stages behind the DMA) ----------
            xr = x_sb.rearrange("p (i ti j tj) -> p i ti j tj", ti=2, tj=2, j=WP)
            v = [xr[:, :, 0, :, 0], xr[:, :, 0, :, 1],
                 xr[:, :, 1, :, 0], xr[:, :, 1, :, 1]]
            m01 = sg1.tile([128, 512], f32)
            m23 = sg1.tile([128, 512], f32)
            pooled = sg1.tile([128, 512], f32, name="pooled")
            m01r = m01.rearrange("p (i j) -> p i j", j=WP)
            m23r = m23.rearrange("p (i j) -> p i j", j=WP)
            pooledr = pooled.rearrange("p (i j) -> p i j", j=WP)
            for half in range(2):
                sl = slice(half * 8, half * 8 + 8)
                nc.vector.tensor_tensor(m01r[:, sl], v[0][:, sl], v[1][:, sl], op=OP.max)
                nc.vector.tensor_tensor(m23r[:, sl], v[2][:, sl], v[3][:, sl], op=OP.max)
                nc.vector.tensor_tensor(pooledr[:, sl], m01r[:, sl], m23r[:, sl], op=OP.max)

            # f32r copy of pooled for the PE (masks need the exact f32)
            pooled_r = sg1.tile([128, 512], f32r)
            nc.vector.tensor_copy(pooled_r, pooled)
            wprojB = sg1.tile([128, 128], bf16)
            nc.vector.tensor_copy(wprojB, wprojP.bitcast(f32))

            # ---------------- prep: warmup + QKV matmuls ----------------
            qtp = [sg1.tile([128, N], f32r, tag=f"qtp{sg}", name=f"qtp{sg}")
                   for sg in range(4)]
            ktp = [sg1.tile([128, N], f32r, tag=f"ktp{sg}", name=f"ktp{sg}")
                   for sg in range(4)]
            with (
                tc.tile_pool(name="prepq", bufs=6, space="PSUM") as prepq,
                tc.tile_pool(name="prepv", bufs=2, space="PSUM") as prepv,
            ):
                for wi in range(9):
                    w_ps = prepq.tile([128, 512], f32, tag="qkps")
                    nc.tensor.matmul(w_ps, dummy_bf[:, 0:128], dummy_bf,
                                     start=True, stop=True)
                qk_ps = {}
                for sg in range(4):
                    for qh in range(2):
                        psl = slice(64 * qh, 64 * qh + 64)
                        tp = (64 * qh, 0)
                        q_ps = prepq.tile([128, 512], f32, tag="qkps")
                        nc.tensor.matmul(q_ps, wqp[sg][psl], pooled_r[psl],
                                         start=True, stop=True, tile_position=tp)
                        k_ps = prepq.tile([128, 512], f32, tag="qkps")
                        nc.tensor.matmul(k_ps, wkp[sg][psl], pooled_r[psl],
                                         start=True, stop=True, tile_position=tp)
                        qk_ps[(sg, qh)] = (q_ps, k_ps)
                    if sg in (0, 2):
                        for kt in range(4 * (sg // 2), 4 * (sg // 2) + 4):
                            qh = kt // 4
                            psl = slice(64 * qh, 64 * qh + 64)
                            v_ps = prepv.tile([128, DIM], f32, tag="vps")
                            nc.tensor.matmul(
                                v_ps, pooled_r[psl, (kt % 4) * 128:(kt % 4) * 128 + 128],
                                wv[psl], start=True, stop=True,
                                tile_position=(64 * qh, 0))
                            nc.vector.tensor_copy(
                                vt[kt][:, :, 0:4],
                                v_ps.rearrange("p (h e) -> p h e", e=4))
                # copies PSUM->SBUF: ACT takes 10, DVE takes 6
                copy_jobs = []
                for sg in range(4):
                    for qh in range(2):
                        q_ps, k_ps = qk_ps[(sg, qh)]
                        qsl = slice(qh * 512, qh * 512 + 512)
                        copy_jobs.append((ktp[sg], qsl, k_ps))
                        copy_jobs.append((qtp[sg], qsl, q_ps))
                for i, (dst, qsl, src) in enumerate(copy_jobs):
                    if i < 10:
                        nc.scalar.copy(dst[:, qsl], src)
                    else:
                        nc.vector.tensor_copy(dst[:, qsl], src)

            # ---------------- masks (emitted lazily into attention) ------
            masks = [None] * 4
            _mask_state = {"step": 0, "nf": None}

            def emit_mask_step():
                s = _mask_state["step"]
                if s >= 10:
                    return
                # steps: eq0,eq1,eq2,eq3, nf0, m1, nf1, m2, nf2, m3
                if s < 4:
                    eq = sg1.tile([128, 512], f32, tag=f"eq{s}", name=f"eq{s}")
                    eqr = eq.rearrange("p (i j) -> p i j", j=WP)
                    nc.vector.tensor_tensor(eqr, v[s], pooledr, op=OP.is_equal)
                    if s == 0:
                        masks[0] = eq
                    else:
                        _mask_state[f"eq{s}"] = eq
                elif s == 4:
                    nf = sg1.tile([128, 512], f32, tag="nf0", name="nf0")
                    nc.vector.tensor_scalar(nf, masks[0], -1.0, 1.0,
                                            op0=OP.mult, op1=OP.add)
                    _mask_state["nf"] = nf
                elif s in (5, 7, 9):
                    p = (s - 3) // 2  # 1,2,3
                    mk = sg1.tile([128, 512], f32, tag=f"mk{p}", name=f"mk{p}")
                    nc.gpsimd.tensor_tensor(mk, _mask_state[f"eq{p}"],
                                            _mask_state["nf"], op=OP.mult)
                    masks[p] = mk
                else:  # 6, 8
                    p = (s - 4) // 2  # 1,2
                    nf2 = sg1.tile([128, 512], f32, tag=f"nf{p}", name=f"nf{p}")
                    nc.vector.tensor_tensor(nf2, _mask_state["nf"], masks[p],
                                            op=OP.subtract)
                    _mask_state["nf"] = nf2
                _mask_state["step"] = s + 1

            # ---------------- attention ----------------
            accs = []
            with (
                tc.tile_pool(name="accp", bufs=1, space="PSUM") as accp,
                tc.tile_pool(name="epool", bufs=8) as epool,
            ):
                for qh in range(2):
                    acc = accp.tile([128, 4, 80], f32, tag=f"acc{qh}",
                                    name=f"acc{qh}")
                    nc.vector.memset(acc, 0.0)
                    accs.append(acc)

                def flush_av(item):
                    sg, c, kt, e = item
                    h = 4 * sg + c
                    for qh in range(2):
                        for qc in range(4):
                            nc.tensor.matmul(
                                accs[qh][:, qc, 5 * h:5 * h + 5],
                                e[:, qh * 512 + qc * 128:qh * 512 + qc * 128 + 128],
                                vt[kt][:, h, :],
                                start=False, stop=(kt == NKT - 1),
                                skip_group_check=True)

                pend = []
                j = 0
                with tc.tile_pool(name="slotp", bufs=3, space="PSUM") as slotp:
                    for sg in range(4):
                        for kt in range(NKT):
                            for c in range(4):
                                slot = slotp.tile([128, 1024], f32, tag="slot")
                                for qh in range(2):
                                    nc.tensor.matmul(
                                        slot[:, qh * 512:qh * 512 + 512],
                                        ktp[sg][32 * c:32 * c + 4,
                                                kt * 128:kt * 128 + 128],
                                        qtp[sg][32 * c:32 * c + 4,
                                                qh * 512:qh * 512 + 512],
                                        start=True, stop=True,
                                        tile_position=(32 * c, 0))
                                e = epool.tile([128, 1024], bf16, tag="e")
                                if assign[j] == "A":
                                    nc.scalar.activation(e, slot, AF.Exp)
                                else:
                                    nc.vector.tensor_scalar(
                                        e.bitcast(i16), slot, SCH_A, SCH_B,
                                        op0=OP.mult, op1=OP.add)
                                pend.append((sg, c, kt, e))
                                if len(pend) > 3:
                                    flush_av(pend.pop(0))
                                if j % 12 == 6:
                                    emit_mask_step()
                                j += 1
                    for item in pend:
                        flush_av(item)
                    while _mask_state["step"] < 10:
                        emit_mask_step()

                # ---------------- tail ----------------
                out_sb = sg1.tile([128, 2048], f32)
                outr = out_sb.rearrange("p (i ti j tj) -> p i ti j tj",
                                        ti=2, tj=2, j=WP)
                recip = sg1.tile([128, 8, 16], f32)
                onorm = sg1.tile([128, 8, 16, 4], bf16)
                yb = sg1.tile([128, 8, 64], bf16)
                y_sb = sg1.tile([128, 512], f32, name="y_sb")
                with tc.tile_pool(name="tailp", bufs=1, space="PSUM") as tailp:
                    y_ps = tailp.tile([128, 512], f32)
                    for g in range(8):
                        qh, qc = g // 4, g % 4
                        accv = accs[qh][:, qc, :].rearrange("p (h f) -> p h f", f=5)
                        nc.vector.reciprocal(recip[:, g], accv[:, :, 4])
                        nc.vector.tensor_tensor(
                            onorm[:, g], accv[:, :, 0:4],
                            recip[:, g].unsqueeze(2).broadcast_to([128, 16, 4]),
                            op=OP.mult)
                        nc.vector.transpose(yb[:, g],
                                            onorm[:, g].rearrange("p h f -> p (h f)"))
                        p0 = 64 * qh
                        for r in range(4):
                            ybb = yb[:, g].rearrange("p (bc f) -> p bc f", f=32)
                            for bc in range(2):
                                nc.tensor.matmul(
                                    y_ps[p0:p0 + 64,
                                         qc * 128 + 32 * r:qc * 128 + 32 * r + 32],
                                    wprojB[32 * r:32 * r + 32,
                                           64 * bc:64 * bc + 64],
                                    ybb[32 * r:32 * r + 32, bc],
                                    start=(bc == 0), stop=(bc == 1),
                                    skip_group_check=True,
                                    tile_position=(32 * r, p0))
                    # BN: y = y_ps * bns + bnb (per-partition scalars) on DVE
                    nc.vector.tensor_scalar(y_sb, y_ps, bns, bnb,
                                            op0=OP.mult, op1=OP.add)
                    yr = y_sb.rearrange("p (i j) -> p i j", j=WP)
                    for p in range(4):
                        mr = masks[p].rearrange("p (i j) -> p i j", j=WP)
                        eng = nc.vector if p % 2 == 0 else nc.gpsimd
                        eng.tensor_tensor(outr[:, :, p // 2, :, p % 2], yr, mr,
                                          op=OP.mult)
                    for q in range(4):
                        psl = slice(64 * (q // 2), 64 * (q // 2) + 64)
                        csl = slice(1024 * (q % 2), 1024 * (q % 2) + 1024)
                        nc.sync.dma_start(out=out_d[psl, csl], in_=out_sb[psl, csl])

    nc.compile()
    return nc


def _host_inputs(x, w_qkv, w_proj, gamma, beta, bn_mean, bn_var):
    """Build the per-core input maps (host-side packing)."""
    wq = np.asarray(w_qkv[:, 0:64], dtype=np.float32) * 0.5  # fold softmax scale
    wk = np.asarray(w_qkv[:, 64:128], dtype=np.float32)
    wv = np.asarray(w_qkv[:, 128:192], dtype=np.float32)
    wqp = np.zeros((4, 64, 128), np.float32)
    wkp = np.zeros((4, 64, 128), np.float32)
    for sg in range(4):
        for c in range(4):
            h = 4 * sg + c
            for d in range(HEAD_DIM):
                wqp[sg][:, 32 * c + d] = wq[:, 4 * h + d]
                wkp[sg][:, 32 * c + d] = wk[:, 4 * h + d]
    # wprojP[32r+a, bc, cch] = wproj[32bc+a, cch] for every row-block r
    wprojP = np.zeros((128, 128), np.float32)
    wp = np.asarray(w_proj, dtype=np.float32)
    for r in range(4):
        for a in range(32):
            wprojP[32 * r + a, 0:64] = wp[a, :]
            wprojP[32 * r + a, 64:128] = wp[32 + a, :]
    inv = gamma / np.sqrt(bn_var + BN_EPS)
    bns = inv.astype(np.float32)
    bnb = (beta - bn_mean * inv).astype(np.float32)

    wb = np.zeros((128, 1218), np.float32)
    for sg in range(4):
        wb[0:64, 128 * sg:128 * sg + 128] = wqp[sg]
        wb[0:64, 512 + 128 * sg:512 + 128 * sg + 128] = wkp[sg]
    wb[0:64, 1024:1088] = wv
    wb[64:128, 0:1088] = wb[0:64, 0:1088]
    wb[:, 1088:1216] = wprojP
    wb[0:64, 1216] = bns
    wb[64:128, 1216] = bns
    wb[0:64, 1217] = bnb
    wb[64:128, 1217] = bnb

    in_maps = []
    xa = np.asarray(x, dtype=np.float32)
    for b in range(B):
        xf = xa[b].reshape(DIM, H * W)
        xfold = np.empty((128, 2048), np.float32)
        xfold[0:64] = xf[:, 0:2048]
        xfold[64:128] = xf[:, 2048:4096]
        in_maps.append({"x": np.ascontiguousarray(xfold), "wb": wb})
    return in_maps


def kernel(x, w_qkv, w_proj, gamma, beta, bn_mean, bn_var):
    from concourse import bass_utils

    if "nc" not in _CACHE:
        _CACHE["nc"] = _build_program()
    nc = _CACHE["nc"]
    in_maps = _host_inputs(
        np.asarray(x), np.asarray(w_qkv), np.asarray(w_proj),
        np.asarray(gamma), np.asarray(beta),
        np.asarray(bn_mean), np.asarray(bn_var))
    res = bass_utils.run_bass_kernel_spmd(nc, in_maps, core_ids=list(range(B)))
    out = np.empty((B, DIM, H * W), np.float32)
    for b in range(B):
        of = res.results[b]["out"]
        out[b, :, 0:2048] = of[0:64]
        out[b, :, 2048:4096] = of[64:128]
    return out.reshape(B, DIM, H, W).astype(np.float32)
